# revision 1
# baseline (speedup 1.0000x reference)
"""Trainium2 Bass kernel for nn_Attention_558345749040.

Reference computation (per batch b, H=8 heads of d=64, S=4096, E=512):
    Q = Q_seq @ WQ ; K = K_seq @ WK ; V = V_seq @ WV      (per-token matmuls)
    A = (Q * K) / 8                                        (elementwise)
    A += -1e12 at head positions j >= V_len[b]             (additive mask)
    softmax over each head's 64-wide feature group
    O = softmax * V, rows s >= Q_len[b] zeroed

Sharding: pure data parallel, batch b -> core b (B == 8 == n_cores).

Device algorithm (per core, token-major [128-token, 512-feature] tiles):
  Q/K projections in float32r (full-rate PE, enough mantissa for the exp),
  V projection in fp16. Host pre-zeroes masked columns of WK and WV, so
  masked positions have K=0 => logits A_j = 0 exactly; the mask-free group
  max is then >= 0 and >= every unmasked logit, making exp(A - M) <= 1 and
  leaving masked positions excluded from the denominator via a 0/1 vmask
  multiply on exp's output (and zeroed in the output via the zeroed WV).
  V_len == 0 cores reproduce the reference's uniform-1/64 softmax via
  WK = 0 with vmask = 1. Q_len row masking rides the V PSUM->SBUF copy as
  a per-partition ACT scale. The elementwise/softmax chain runs on wide
  [128, 1024] tiles (two token chunks per instruction) to amortize per-op
  overheads; matmul/PSUM stages stay per-chunk (PSUM bank budget).
"""

import numpy as np
import ml_dtypes

B, S, EMB = 8, 4096, 512
H, D = 8, 64
NCORES = 8
KC = EMB // 128          # 4 contraction chunks
NCHUNK = S // 128        # 32 token chunks
SUP = 8                  # token chunks per super-chunk (input DMA granularity)
NSUP = NCHUNK // SUP
W = 2                    # token chunks per wide elementwise tile

_CACHE = {}


def _build(cfg=""):
    import concourse.bacc as bacc
    import concourse.mybir as mybir
    from concourse.tile import TileContext

    f32 = mybir.dt.float32
    f32r = mybir.dt.float32r
    f16 = mybir.dt.float16
    bf16 = mybir.dt.bfloat16
    AX = mybir.AxisListType
    OP = mybir.AluOpType
    ACTF = mybir.ActivationFunctionType

    nc = bacc.Bacc()

    WE = W * EMB
    qT = nc.declare_dram_parameter("qT", [EMB, S], f32r, isOutput=False)
    kT = nc.declare_dram_parameter("kT", [EMB, S], f32r, isOutput=False)
    vT = nc.declare_dram_parameter("vT", [EMB, S], f16, isOutput=False)
    wq = nc.declare_dram_parameter("wq", [EMB, EMB], f32r, isOutput=False)
    wk = nc.declare_dram_parameter("wk", [EMB, EMB], f32r, isOutput=False)
    wv = nc.declare_dram_parameter("wv", [EMB, EMB], f16, isOutput=False)
    vmask = nc.declare_dram_parameter("vmask", [128, WE], bf16, isOutput=False)
    qmask = nc.declare_dram_parameter("qmask", [128, NCHUNK], f32, isOutput=False)
    out = nc.declare_dram_parameter("out", [S, EMB], bf16, isOutput=True)

    def view_hd(ap):
        # [128, W*EMB] -> [128, W*H, D]
        return ap.rearrange("p (g d) -> p g d", d=D)

    def bcast_hd(ap):
        # [128, W*H] -> [128, W*H, D] with step-0 broadcast
        return ap.rearrange("p (g o) -> p g o", o=1).broadcast_to((128, W * H, D))

    with TileContext(nc) as tc:
        with (
            tc.tile_pool(name="consts", bufs=1) as cpool,
            tc.tile_pool(name="xin", bufs=2) as xpool,
            tc.tile_pool(name="ps", bufs=2, space="PSUM") as ppool,
            tc.tile_pool(name="psq3", bufs=3, space="PSUM") as qpool,
            tc.tile_pool(name="work", bufs=3) as wpool,
            tc.tile_pool(name="live", bufs=4) as lpool,
            tc.tile_pool(name="stats", bufs=4) as spool,
        ):
            w_sb = {}
            for name, src, dt_ in (("wq", wq, f32r), ("wk", wk, f32r),
                                   ("wv", wv, f16)):
                tiles = []
                for kc in range(KC):
                    t = cpool.tile([128, EMB], dt_, tag=f"{name}{kc}",
                                   name=f"{name}{kc}")
                    nc.sync.dma_start(out=t[:], in_=src[kc * 128:(kc + 1) * 128, :])
                    tiles.append(t)
                w_sb[name] = tiles
            vm_sb = cpool.tile([128, WE], bf16, tag="vmask")
            nc.sync.dma_start(out=vm_sb[:], in_=vmask[:, :])
            qm_sb = cpool.tile([128, NCHUNK], f32, tag="qm")
            nc.sync.dma_start(out=qm_sb[:], in_=qmask[:, :])

            npairs = NCHUNK // W

            def load_sup(s, split=1):
                # split>1: issue the load in `split` column slices so the
                # first pair's data lands early (kills the startup ramp).
                tok0 = s * SUP * 128
                cols = SUP * 128
                xs = {}
                tiles = {}
                for name, src, dt_ in (("q", qT, f32r), ("k", kT, f32r),
                                       ("v", vT, f16)):
                    tiles[name] = [xpool.tile([128, cols], dt_, tag=f"x{name}{kc}",
                                              name=f"x{name}{kc}")
                                   for kc in range(KC)]
                    xs[name] = tiles[name]
                for part in range(split):
                    c0, c1 = part * cols // split, (part + 1) * cols // split
                    for name, src, dt_ in (("q", qT, f32r), ("k", kT, f32r),
                                           ("v", vT, f16)):
                        for kc in range(KC):
                            nc.sync.dma_start(
                                out=tiles[name][kc][:, c0:c1],
                                in_=src[kc * 128:(kc + 1) * 128,
                                        tok0 + c0:tok0 + c1],
                            )
                return xs

            def stage_front(pair, xs):
                # matmuls, PSUM copies, logits, group max, max-subtract, exp
                k_sb = wpool.tile([128, WE], f32, tag="k_sb")
                v_sb = lpool.tile([128, WE], bf16, tag="v_sb")
                a = wpool.tile([128, WE], f32, tag="a")
                psvs = []
                for c in range(W):
                    chunk = pair * W + c
                    j = chunk % SUP
                    js = slice(j * 128, (j + 1) * 128)
                    cs = slice(c * EMB, (c + 1) * EMB)
                    psq = qpool.tile([128, EMB], f32, tag="psq")
                    psk = ppool.tile([128, EMB], f32, tag="psk")
                    for name, ps, wn in (("k", psk, "wk"), ("q", psq, "wq")):
                        for kc in range(KC):
                            nc.tensor.matmul(
                                ps[:],
                                xs[name][kc][:, js],
                                w_sb[wn][kc][:],
                                start=(kc == 0),
                                stop=(kc == KC - 1),
                            )
                    nc.scalar.copy(k_sb[:, cs], psk[:])
                    nc.vector.tensor_mul(a[:, cs], psq[:], k_sb[:, cs])
                for c in range(W):
                    chunk = pair * W + c
                    j = chunk % SUP
                    js = slice(j * 128, (j + 1) * 128)
                    cs = slice(c * EMB, (c + 1) * EMB)
                    psv = ppool.tile([128, EMB], f32, tag="psv", bufs=3)
                    for kc in range(KC):
                        nc.tensor.matmul(
                            psv[:],
                            xs["v"][kc][:, js],
                            w_sb["wv"][kc][:],
                            start=(kc == 0),
                            stop=(kc == KC - 1),
                        )
                    nc.scalar.activation(
                        v_sb[:, cs], psv[:], ACTF.Copy,
                        scale=qm_sb[:, chunk:chunk + 1],
                    )
                mneg = spool.tile([128, W * H], f32, tag="mneg")
                nc.vector.tensor_reduce(
                    mneg[:], view_hd(a[:]), axis=AX.X, op=OP.max, negate=True
                )
                t_m = wpool.tile([128, WE], f32, tag="t_m")
                nc.gpsimd.tensor_add(
                    view_hd(t_m[:]), view_hd(a[:]), bcast_hd(mneg[:])
                )
                e = lpool.tile([128, WE], bf16, tag="e")
                nc.scalar.activation(e[:], t_m[:], ACTF.Exp)
                return e, v_sb

            def stage_back(pair, e, v_sb):
                # denominator, reciprocal, normalize, weight V, store
                em = wpool.tile([128, WE], bf16, tag="em")
                nc.vector.tensor_mul(em[:], e[:], vm_sb[:])
                ssum = spool.tile([128, W * H], f32, tag="ssum")
                nc.vector.tensor_reduce(
                    ssum[:], view_hd(em[:]), axis=AX.X, op=OP.add
                )
                r = spool.tile([128, W * H], bf16, tag="r")
                with nc.allow_low_precision(reason="1/S at bf16: 0.4% on softmax weights, well under the 2e-2 gate"):
                    nc.vector.reciprocal(r[:], ssum[:])
                p = wpool.tile([128, WE], bf16, tag="p")
                nc.gpsimd.tensor_mul(
                    view_hd(p[:]), view_hd(em[:]), bcast_hd(r[:])
                )
                o = wpool.tile([128, WE], bf16, tag="o")
                nc.vector.tensor_mul(o[:], p[:], v_sb[:])
                t0 = pair * W * 128
                nc.sync.dma_start(
                    out=out[t0:t0 + W * 128, :].rearrange("(i p) f -> p i f", i=W),
                    in_=o[:].rearrange("p (i f) -> p i f", i=W),
                )

            pairs_per_sup = SUP // W
            xs_cur = load_sup(0, split=1)
            xs_next = None
            pending = None
            for pair in range(npairs + 1):
                if pair < npairs:
                    s, local = divmod(pair, pairs_per_sup)
                    if local == 0 and s > 0:
                        xs_cur = load_sup(s)
                    front = stage_front(pair, xs_cur)
                else:
                    front = None
                if pending is not None:
                    stage_back(pair - 1, *pending)
                pending = front

    nc.finalize()
    return nc


def _prep_inputs(Q_seq, K_seq, V_seq, Q_len, V_len, WQ, WK, WV):
    in_maps = []
    jpos = np.arange(EMB) % D
    tpos = np.arange(S)
    for b in range(B):
        vl = int(V_len[b, 0])
        ql = int(Q_len[b, 0])
        if vl == 0:
            # Reference semantics collapse to a uniform 1/64 softmax (every
            # logit rides to exactly -1e12 in f32). Reproduce via K = 0
            # (all logits 0 -> uniform) with every position unmasked.
            wk_b = np.zeros_like(WK, dtype=np.float32)
            wv_b = WV.astype(np.float32)
            vmrow = np.ones(EMB, np.float32)
        else:
            keep = (jpos < vl)
            wk_b = np.where(keep[None, :], WK, 0.0).astype(np.float32)
            wv_b = np.where(keep[None, :], WV, 0.0).astype(np.float32)
            vmrow = keep.astype(np.float32)
        vmrow_w = np.tile(vmrow, W).astype(ml_dtypes.bfloat16)
        vmask = np.broadcast_to(vmrow_w, (128, W * EMB)).copy()
        qm = (tpos < ql).astype(np.float32).reshape(NCHUNK, 128).T.copy()
        in_maps.append({
            "qT": np.ascontiguousarray(Q_seq[b].T.astype(np.float32)),
            "kT": np.ascontiguousarray(K_seq[b].T.astype(np.float32)),
            "vT": np.ascontiguousarray(V_seq[b].T.astype(np.float16)),
            "wq": np.ascontiguousarray((WQ * 0.125).astype(np.float32)),
            "wk": np.ascontiguousarray(wk_b),
            "wv": np.ascontiguousarray(wv_b.astype(np.float16)),
            "vmask": vmask,
            "qmask": np.ascontiguousarray(qm),
        })
    return in_maps


def _run(inputs, trace=False, mm_dtype_name="", tmpdir=None):
    from concourse.bass_utils import run_bass_kernel_spmd

    key = "v7"
    if key not in _CACHE:
        _CACHE[key] = _build()
    nc = _CACHE[key]

    in_maps = _prep_inputs(**inputs)
    res = run_bass_kernel_spmd(nc, in_maps, core_ids=list(range(NCORES)),
                               trace=trace, tmpdir=tmpdir)
    out = np.stack([res.results[i]["out"] for i in range(NCORES)], axis=0)
    return out.astype(np.float32), res


def kernel(Q_seq, K_seq, V_seq, Q_len, V_len, WQ, WK, WV):
    out, _ = _run(dict(Q_seq=Q_seq, K_seq=K_seq, V_seq=V_seq,
                       Q_len=Q_len, V_len=V_len, WQ=WQ, WK=WK, WV=WV))
    return out



# revision 4
# speedup vs baseline: 1.2984x; 1.2984x over previous
"""Trainium2 Bass kernel for nn_Attention_558345749040.

Reference (per batch b, H=8 heads of d=64, S=4096, E=512):
    Q = Q_seq @ WQ ; K = K_seq @ WK ; V = V_seq @ WV
    A = (Q * K) / 8                      (elementwise)
    softmax over each head's 64-wide feature group, positions j >= V_len[b]
    masked out (V_len == 0 degenerates to a uniform 1/64 softmax)
    O = softmax * V, rows s >= Q_len[b] zeroed

Structure exploited (all derived from the runtime Q_len / V_len values, so
the compiled schedule is input-shape-specialized but value-generic):
  * Rows s >= Q_len[b] are zero: only ceil(Q_len/128) 128-token chunks per
    batch carry live data. The live chunks of all batches are repartitioned
    evenly across the 8 cores (token-balanced data parallel) instead of
    batch-per-core, which removes the Q_len imbalance entirely.
  * Only head positions j < V_len[b] contribute: the Q/K/V matmuls select
    just the 8*V_len live weight columns through a strided moving AP over
    the shared full weight tiles (PE matmul cost scales with output free
    size, so narrow slots are proportionally cheaper), the softmax runs on
    vl-wide groups with no masking at all, and only the packed columns are
    stored; the host scatters them back into the zero canvas.
  * V_len == 0 batches reduce to O = V/64: V-matmul-only slots (the 1/64
    and the Q_len row mask ride the PSUM->SBUF copy as a per-row scale).
  * fp16 transport + fp16 matmuls throughout (measured rel err 3.6e-3 vs
    the 2e-2 gate; bf16 Q/K fails at 2.5e-2, fp8 V fails at 3.7e-2).

Every core runs the same instruction stream (SPMD single-NEFF constraint):
the slot schedule (widths/kinds) is identical across cores; which batch
chunk a slot processes is pure data (gathered inputs + per-slot masks).
"""

import numpy as np
import ml_dtypes

B, S, EMB = 8, 4096, 512
H, D = 8, 64
NCORES = 8
KC = EMB // 128          # 4 contraction chunks
SUP = 8                  # slots per input-DMA superslot

_CACHE = {}


def _plan(Q_len, V_len):
    """Slot schedule shared by all cores + per-core chunk assignment.

    Returns (slots, assign) where slots is a list of dicts
    {kind: 'reg'|'vonly', b, c, L, off} (off = column offset in the packed
    output) and assign[i][j] = tok0 of the chunk core i processes in slot j
    (None = dummy slot, output ignored).
    """
    slots = []
    assign = [[] for _ in range(NCORES)]

    def add_batch(b, kind, c):
        ql = int(Q_len[b, 0])
        nch = -(-ql // 128) if ql > 0 else 0
        if nch == 0:
            return
        quota = -(-nch // NCORES)
        for t in range(quota):
            slots.append({"kind": kind, "b": b, "c": c, "L": 8 * c})
            for i in range(NCORES):
                ch = t * NCORES + i
                assign[i].append(ch * 128 if ch < nch else None)

    # V-only slots first: they need only WV + V data, so the PE starts
    # within ~2us of launch while Q/K weights and inputs stream in.
    for b in range(B):
        if int(V_len[b, 0]) == 0:
            add_batch(b, "vonly", 64)
    for b in range(B):
        vl = int(V_len[b, 0])
        if vl > 0:
            add_batch(b, "reg", vl)

    off = 0
    for s in slots:
        s["off"] = off
        off += s["L"]
    return slots, assign, off


def _build(slots, total_L):
    import concourse.bacc as bacc
    import concourse.mybir as mybir
    from concourse.tile import TileContext

    f32 = mybir.dt.float32
    f16 = mybir.dt.float16
    bf16 = mybir.dt.bfloat16
    AX = mybir.AxisListType
    OP = mybir.AluOpType
    ACTF = mybir.ActivationFunctionType

    nslot = len(slots)
    reg_idx = [j for j, s in enumerate(slots) if s["kind"] == "reg"]
    von_idx = [j for j, s in enumerate(slots) if s["kind"] == "vonly"]

    nc = bacc.Bacc()

    qg = nc.declare_dram_parameter("qg", [EMB, nslot * 128], f16, isOutput=False)
    kg = nc.declare_dram_parameter("kg", [EMB, nslot * 128], f16, isOutput=False)
    vg = nc.declare_dram_parameter("vg", [EMB, nslot * 128], f16, isOutput=False)
    wq = nc.declare_dram_parameter("wq", [EMB, EMB], f16, isOutput=False)
    wk = nc.declare_dram_parameter("wk", [EMB, EMB], f16, isOutput=False)
    wv = nc.declare_dram_parameter("wv", [EMB, EMB], f16, isOutput=False)
    qm = nc.declare_dram_parameter("qm", [128, nslot], f32, isOutput=False)
    outp = nc.declare_dram_parameter("outp", [128, total_L], bf16, isOutput=True)

    nsup = -(-nslot // SUP)

    with TileContext(nc) as tc:
        with (
            tc.tile_pool(name="consts", bufs=1) as cpool,
            tc.tile_pool(name="xin", bufs=2) as xpool,
            tc.tile_pool(name="psq2", bufs=2, space="PSUM") as qpool,
            tc.tile_pool(name="psk2", bufs=2, space="PSUM") as kpool,
            tc.tile_pool(name="psv3", bufs=3, space="PSUM") as vpool,
            tc.tile_pool(name="work", bufs=3) as wpool,
            tc.tile_pool(name="live", bufs=4) as lpool,
            tc.tile_pool(name="outs", bufs=3) as opool,
            tc.tile_pool(name="stats", bufs=4) as spool,
        ):
            qm_sb = cpool.tile([128, nslot], f32, tag="qm")
            nc.sync.dma_start(out=qm_sb[:], in_=qm[:, :])

            # Weights: WV first (V-only slots run first), then WK/WQ.
            w_sb = {}
            for name, src in (("wv", wv), ("wk", wk), ("wq", wq)):
                tiles = []
                for kc in range(KC):
                    t = cpool.tile([128, EMB], f16, tag=f"{name}{kc}",
                                   name=f"{name}{kc}")
                    nc.sync.dma_start(out=t[:], in_=src[kc * 128:(kc + 1) * 128, :])
                    tiles.append(t)
                w_sb[name] = tiles

            def w_ap(name, kc, c):
                if c == 64:
                    return w_sb[name][kc][:]
                return (w_sb[name][kc][:]
                        .rearrange("p (h j) -> p h j", j=D)[:, :, :c])

            nvon = len(von_idx)  # vonly slots sit at the front of the order

            def load_sup(sup, split=1):
                lo, hi = sup * SUP, min((sup + 1) * SUP, nslot)
                tiles = {}
                for name in ("v", "k", "q"):
                    tiles[name] = [
                        xpool.tile([128, SUP * 128], f16, tag=f"x{name}{kc}",
                                   name=f"x{name}{kc}")
                        for kc in range(KC)
                    ]
                for part in range(split):
                    for name, src in (("v", vg), ("k", kg), ("q", qg)):
                        # q/k are never read by vonly slots; skip their cols
                        lo_n = lo if name == "v" else max(lo, nvon)
                        if lo_n >= hi:
                            continue
                        cols = (hi - lo_n) * 128
                        c0 = part * cols // split
                        c1 = (part + 1) * cols // split
                        if c0 == c1:
                            continue
                        tcol = (lo_n - lo) * 128
                        for kc in range(KC):
                            nc.sync.dma_start(
                                out=tiles[name][kc][:, tcol + c0:tcol + c1],
                                in_=src[kc * 128:(kc + 1) * 128,
                                        lo_n * 128 + c0:lo_n * 128 + c1],
                            )
                return tiles

            def x_ap(tiles, name, kc, j):
                col = (j % SUP) * 128
                return tiles[name][kc][:, col:col + 128]

            def front(j, xs):
                s = slots[j]
                c, L, kind = s["c"], s["L"], s["kind"]
                psv = vpool.tile([128, EMB], f32, tag="psv")
                for kc in range(KC):
                    nc.tensor.matmul(
                        psv[:, :L], x_ap(xs, "v", kc, j), w_ap("wv", kc, c),
                        start=(kc == 0), stop=(kc == KC - 1),
                    )
                if kind == "vonly":
                    o = opool.tile([128, EMB], bf16, tag="o")
                    nc.scalar.activation(
                        o[:, :L], psv[:, :L], ACTF.Copy,
                        scale=qm_sb[:, j:j + 1],
                    )
                    t0 = s["off"]
                    nc.sync.dma_start(out=outp[:, t0:t0 + L], in_=o[:, :L])
                    return None
                psk = kpool.tile([128, EMB], f32, tag="psk")
                psq = qpool.tile([128, EMB], f32, tag="psq")
                for ps, xn, wn in ((psk, "k", "wk"), (psq, "q", "wq")):
                    for kc in range(KC):
                        nc.tensor.matmul(
                            ps[:, :L], x_ap(xs, xn, kc, j), w_ap(wn, kc, c),
                            start=(kc == 0), stop=(kc == KC - 1),
                        )
                # DVE may read at most one PSUM operand: stage K through SBUF
                k_sb = wpool.tile([128, EMB], f32, tag="k_sb")
                nc.scalar.copy(k_sb[:, :L], psk[:, :L])
                a = wpool.tile([128, EMB], f32, tag="a")
                nc.vector.tensor_mul(a[:, :L], psq[:, :L], k_sb[:, :L])
                mneg = spool.tile([128, H], f32, tag="mneg")
                av = a[:, :L].rearrange("p (g j) -> p g j", j=c)
                nc.vector.tensor_reduce(mneg[:], av, axis=AX.X, op=OP.max,
                                        negate=True)
                t_m = wpool.tile([128, EMB], f32, tag="t_m")
                mneg_b = (mneg[:].rearrange("p (g o) -> p g o", o=1)
                          .broadcast_to((128, H, c)))
                nc.gpsimd.tensor_add(
                    t_m[:, :L].rearrange("p (g j) -> p g j", j=c), av, mneg_b)
                e = lpool.tile([128, EMB], bf16, tag="e")
                nc.scalar.activation(e[:, :L], t_m[:, :L], ACTF.Exp)
                v_sb = lpool.tile([128, EMB], f16, tag="v_sb")
                nc.scalar.activation(
                    v_sb[:, :L], psv[:, :L], ACTF.Copy,
                    scale=qm_sb[:, j:j + 1],
                )
                return e, v_sb

            def back(j, e, v_sb):
                s = slots[j]
                c, L = s["c"], s["L"]
                ev = e[:, :L].rearrange("p (g j) -> p g j", j=c)
                ssum = spool.tile([128, H], f32, tag="ssum")
                nc.vector.tensor_reduce(ssum[:], ev, axis=AX.X, op=OP.add)
                r = spool.tile([128, H], bf16, tag="r")
                with nc.allow_low_precision(reason="1/S at bf16: ~0.4% on softmax weights, well under the 2e-2 gate"):
                    nc.vector.reciprocal(r[:], ssum[:])
                p = wpool.tile([128, EMB], bf16, tag="p")
                r_b = (r[:].rearrange("p (g o) -> p g o", o=1)
                       .broadcast_to((128, H, c)))
                nc.gpsimd.tensor_mul(
                    p[:, :L].rearrange("p (g j) -> p g j", j=c), ev, r_b)
                o = opool.tile([128, EMB], bf16, tag="o")
                nc.vector.tensor_mul(o[:, :L], p[:, :L], v_sb[:, :L])
                t0 = s["off"]
                nc.sync.dma_start(out=outp[:, t0:t0 + L], in_=o[:, :L])

            xs_cur = load_sup(0, split=4)
            pending = None
            for j in range(nslot + 1):
                if j < nslot:
                    sup, local = divmod(j, SUP)
                    if local == 0 and sup > 0:
                        xs_cur = load_sup(sup)
                    res = front(j, xs_cur)
                else:
                    res = None
                if pending is not None:
                    back(pending[0], *pending[1])
                pending = (j, res) if res is not None else None

    nc.finalize()
    return nc


def _prep_inputs(Q_seq, K_seq, V_seq, Q_len, V_len, WQ, WK, WV):
    slots, assign, total_L = _plan(Q_len, V_len)
    f16 = np.float16
    bf = ml_dtypes.bfloat16
    nslot = len(slots)

    wq_h = np.ascontiguousarray((WQ * 0.125).astype(f16))
    wk_h = np.ascontiguousarray(WK.astype(f16))
    wv_h = np.ascontiguousarray(WV.astype(f16))

    # per-batch transposed fp16 inputs, shared across cores
    qT = {}
    kT = {}
    vT = {}
    for s in slots:
        b = s["b"]
        if b not in vT:
            vT[b] = np.ascontiguousarray(V_seq[b].T.astype(f16))
            if s["kind"] == "reg" or any(
                    t["b"] == b and t["kind"] == "reg" for t in slots):
                qT[b] = np.ascontiguousarray(Q_seq[b].T.astype(f16))
                kT[b] = np.ascontiguousarray(K_seq[b].T.astype(f16))

    in_maps = []
    for i in range(NCORES):
        qg = np.zeros((EMB, nslot * 128), f16)
        kg = np.zeros((EMB, nslot * 128), f16)
        vg = np.zeros((EMB, nslot * 128), f16)
        qmv = np.zeros((128, nslot), np.float32)
        for j, s in enumerate(slots):
            tok0 = assign[i][j]
            if tok0 is None:
                continue
            b = s["b"]
            cs = slice(j * 128, (j + 1) * 128)
            ts = slice(tok0, tok0 + 128)
            vg[:, cs] = vT[b][:, ts]
            ql = int(Q_len[b, 0])
            live = np.clip(ql - tok0, 0, 128)
            scale = (1.0 / 64) if s["kind"] == "vonly" else 1.0
            qmv[:live, j] = scale
            if s["kind"] == "reg":
                qg[:, cs] = qT[b][:, ts]
                kg[:, cs] = kT[b][:, ts]
        in_maps.append({
            "qg": qg, "kg": kg, "vg": vg,
            "wq": wq_h, "wk": wk_h, "wv": wv_h,
            "qm": np.ascontiguousarray(qmv),
        })
    return in_maps, slots, assign, total_L


def _run(inputs, trace=False, mm_dtype_name="", tmpdir=None):
    from concourse.bass_utils import run_bass_kernel_spmd

    Q_len = np.asarray(inputs["Q_len"])
    V_len = np.asarray(inputs["V_len"])
    in_maps, slots, assign, total_L = _prep_inputs(
        np.asarray(inputs["Q_seq"]), np.asarray(inputs["K_seq"]),
        np.asarray(inputs["V_seq"]), Q_len, V_len,
        np.asarray(inputs["WQ"]), np.asarray(inputs["WK"]),
        np.asarray(inputs["WV"]))

    key = tuple((s["kind"], s["L"]) for s in slots)
    if key not in _CACHE:
        _CACHE[key] = _build(slots, total_L)
    nc = _CACHE[key]

    res = run_bass_kernel_spmd(nc, in_maps, core_ids=list(range(NCORES)),
                               trace=trace, tmpdir=tmpdir)

    out = np.zeros((B, S, H * D), np.float32)
    for i in range(NCORES):
        po = res.results[i]["outp"].astype(np.float32)
        for j, s in enumerate(slots):
            tok0 = assign[i][j]
            if tok0 is None:
                continue
            b, c, L, off = s["b"], s["c"], s["L"], s["off"]
            block = po[:, off:off + L].reshape(128, H, c)
            out[b, tok0:tok0 + 128].reshape(128, H, D)[:, :, :c] = block
    return out, res


def kernel(Q_seq, K_seq, V_seq, Q_len, V_len, WQ, WK, WV):
    out, _ = _run(dict(Q_seq=Q_seq, K_seq=K_seq, V_seq=V_seq,
                       Q_len=Q_len, V_len=V_len, WQ=WQ, WK=WK, WV=WV))
    return out


# revision 9
# speedup vs baseline: 1.7453x; 1.3442x over previous
"""Trainium2 Bass kernel for nn_Attention_558345749040.

Reference (per batch b, H=8 heads of d=64, S=4096, E=512):
    Q = Q_seq @ WQ ; K = K_seq @ WK ; V = V_seq @ WV
    A = (Q * K) / 8                      (elementwise)
    softmax over each head's 64-wide feature group, positions j >= V_len[b]
    masked out (V_len == 0 degenerates to a uniform 1/64 softmax)
    O = softmax * V, rows s >= Q_len[b] zeroed

Structure exploited (all derived from the runtime Q_len / V_len values, so
the compiled schedule is input-shape-specialized but value-generic):
  * Rows s >= Q_len[b] are zero: only ceil(Q_len/128) 128-token chunks per
    batch carry live data. The live chunks of all batches are repartitioned
    evenly across the 8 cores (token-balanced data parallel) instead of
    batch-per-core, which removes the Q_len imbalance entirely.
  * Only head positions j < V_len[b] contribute: the Q/K/V matmuls select
    just the 8*V_len live weight columns through a strided moving AP over
    the shared full weight tiles (PE matmul cost scales with output free
    size, so narrow slots are proportionally cheaper), the softmax runs on
    vl-wide groups with no masking at all, and only the packed columns are
    stored; the host scatters them back into the zero canvas.
  * V_len == 0 batches reduce to O = V/64: V-matmul-only slots (the 1/64
    and the Q_len row mask ride the PSUM->SBUF copy as a per-row scale).
  * fp16 transport + fp16 matmuls throughout (measured rel err 3.6e-3 vs
    the 2e-2 gate; bf16 Q/K fails at 2.5e-2, fp8 V fails at 3.7e-2).

Every core runs the same instruction stream (SPMD single-NEFF constraint):
the slot schedule (widths/kinds) is identical across cores; which batch
chunk a slot processes is pure data (gathered inputs + per-slot masks).
"""

import numpy as np
import ml_dtypes

B, S, EMB = 8, 4096, 512
H, D = 8, 64
NCORES = 8
KC = EMB // 128          # 4 contraction chunks
SUP = 8                  # slots per input-DMA superslot

_CACHE = {}


def _plan(Q_len, V_len):
    """Slot schedule shared by all cores + per-core chunk assignment.

    Returns (slots, assign) where slots is a list of dicts
    {kind: 'reg'|'vonly', b, c, L, off} (off = column offset in the packed
    output) and assign[i][j] = tok0 of the chunk core i processes in slot j
    (None = dummy slot, output ignored).
    """
    slots = []
    assign = [[] for _ in range(NCORES)]

    def add_batch(b, kind, c):
        ql = int(Q_len[b, 0])
        nch = -(-ql // 128) if ql > 0 else 0
        if nch == 0:
            return
        quota = -(-nch // NCORES)
        for t in range(quota):
            slots.append({"kind": kind, "b": b, "c": c, "L": 8 * c})
            for i in range(NCORES):
                ch = t * NCORES + i
                assign[i].append(ch * 128 if ch < nch else None)

    # V-only slots first: they need only WV + V data, so the PE starts
    # within ~2us of launch while Q/K weights and inputs stream in.
    for b in range(B):
        if int(V_len[b, 0]) == 0:
            add_batch(b, "vonly", 64)
    for b in range(B):
        vl = int(V_len[b, 0])
        if vl > 0:
            add_batch(b, "reg", vl)

    off = 0
    for s in slots:
        s["off"] = off
        off += s["L"]
    return slots, assign, off


def _build(slots, total_L):
    import concourse.bacc as bacc
    import concourse.mybir as mybir
    from concourse.tile import TileContext

    f32 = mybir.dt.float32
    f16 = mybir.dt.float16
    bf16 = mybir.dt.bfloat16
    AX = mybir.AxisListType
    OP = mybir.AluOpType
    ACTF = mybir.ActivationFunctionType

    nslot = len(slots)
    reg_idx = [j for j, s in enumerate(slots) if s["kind"] == "reg"]
    von_idx = [j for j, s in enumerate(slots) if s["kind"] == "vonly"]

    nc = bacc.Bacc()

    qg = nc.declare_dram_parameter("qg", [EMB, nslot * 128], f16, isOutput=False)
    kg = nc.declare_dram_parameter("kg", [EMB, nslot * 128], f16, isOutput=False)
    vg = nc.declare_dram_parameter("vg", [EMB, nslot * 128], f16, isOutput=False)
    wq = nc.declare_dram_parameter("wq", [EMB, EMB], f16, isOutput=False)
    wk = nc.declare_dram_parameter("wk", [EMB, EMB], f16, isOutput=False)
    wv = nc.declare_dram_parameter("wv", [EMB, EMB], f16, isOutput=False)
    qm = nc.declare_dram_parameter("qm", [128, nslot], f32, isOutput=False)
    outp = nc.declare_dram_parameter("outp", [128, total_L], bf16, isOutput=True)

    nsup = -(-nslot // SUP)

    with TileContext(nc) as tc:
        with (
            tc.tile_pool(name="consts", bufs=1) as cpool,
            tc.tile_pool(name="xin", bufs=2) as xpool,
            tc.tile_pool(name="psq2", bufs=2, space="PSUM") as qpool,
            tc.tile_pool(name="psk2", bufs=2, space="PSUM") as kpool,
            tc.tile_pool(name="psv3", bufs=3, space="PSUM") as vpool,
            tc.tile_pool(name="work", bufs=3) as wpool,
            tc.tile_pool(name="live", bufs=4) as lpool,
            tc.tile_pool(name="outs", bufs=3) as opool,
            tc.tile_pool(name="stats", bufs=4) as spool,
        ):
            qm_sb = cpool.tile([128, nslot], f32, tag="qm")
            nc.sync.dma_start(out=qm_sb[:], in_=qm[:, :])

            # Weights: one DMA trigger per tensor (HWDGE triggers cost a
            # flat ~625ns on a shared generator — count is what matters).
            # Layout: kc-major blocks of [128, EMB] in a single tile.
            w_sb = {}
            for name, src in (("wv", wv), ("wk", wk), ("wq", wq)):
                t = cpool.tile([128, KC * EMB], f16, tag=name, name=name)
                nc.sync.dma_start(
                    out=t[:].rearrange("p (k c) -> p k c", k=KC),
                    in_=src[:, :].rearrange("(k p) c -> p k c", p=128),
                )
                w_sb[name] = t

            def w_ap(name, kc, c):
                blk = w_sb[name][:, kc * EMB:(kc + 1) * EMB]
                if c == 64:
                    return blk
                return blk.rearrange("p (h j) -> p h j", j=D)[:, :, :c]

            nvon = len(von_idx)  # vonly slots sit at the front of the order
            SUPW = SUP * 128

            def load_sup(sup, split=1):
                # One DMA trigger per (sup, tensor): all 4 kc chunks land in
                # a single kc-major tile via a 3D access pattern.
                lo, hi = sup * SUP, min((sup + 1) * SUP, nslot)
                tiles = {}
                for name in ("v", "k", "q"):
                    tiles[name] = xpool.tile([128, KC * SUPW], f16,
                                             tag=f"x{name}", name=f"x{name}")
                for name, src in (("v", vg), ("k", kg), ("q", qg)):
                    # q/k are never read by vonly slots; skip their cols
                    lo_n = lo if name == "v" else max(lo, nvon)
                    if lo_n >= hi:
                        continue
                    cols = (hi - lo_n) * 128
                    tcol = (lo_n - lo) * 128
                    dst = tiles[name][:].rearrange(
                        "p (k c) -> p k c", k=KC)[:, :, tcol:tcol + cols]
                    for part in range(split):
                        c0 = part * cols // split
                        c1 = (part + 1) * cols // split
                        if c0 == c1:
                            continue
                        nc.sync.dma_start(
                            out=dst[:, :, c0:c1],
                            in_=src[:, lo_n * 128 + c0:lo_n * 128 + c1]
                            .rearrange("(k p) c -> p k c", p=128),
                        )
                return tiles

            def x_ap(tiles, name, kc, j):
                col = kc * SUPW + (j % SUP) * 128
                return tiles[name][:, col:col + 128]

            # Output staging: one [128, supL] tile per superslot, a single
            # store trigger per sup issued from the ACT queue (keeps the
            # sync queue free for input loads).
            sup_lo = [sp * SUP for sp in range(nsup)]
            sup_hi = [min((sp + 1) * SUP, nslot) for sp in range(nsup)]
            supL = [sum(slots[j]["L"] for j in range(sup_lo[sp], sup_hi[sp]))
                    for sp in range(nsup)]
            oloc = {}
            for sp in range(nsup):
                col = 0
                for j in range(sup_lo[sp], sup_hi[sp]):
                    oloc[j] = col
                    col += slots[j]["L"]
            max_supL = max(supL)
            otiles = {}

            def o_slice(j):
                sp = j // SUP
                if sp not in otiles:
                    otiles[sp] = opool.tile([128, max_supL], bf16, tag="osup",
                                            name="osup")
                return otiles[sp][:, oloc[j]:oloc[j] + slots[j]["L"]]

            def store_sup(sp):
                g0 = slots[sup_lo[sp]]["off"]
                nc.scalar.dma_start(out=outp[:, g0:g0 + supL[sp]],
                                    in_=otiles[sp][:, :supL[sp]])

            def front(j, xs):
                s = slots[j]
                c, L, kind = s["c"], s["L"], s["kind"]
                psv = vpool.tile([128, EMB], f32, tag="psv")
                for kc in range(KC):
                    nc.tensor.matmul(
                        psv[:, :L], x_ap(xs, "v", kc, j), w_ap("wv", kc, c),
                        start=(kc == 0), stop=(kc == KC - 1),
                    )
                if kind == "vonly":
                    nc.scalar.activation(
                        o_slice(j), psv[:, :L], ACTF.Copy,
                        scale=qm_sb[:, j:j + 1],
                    )
                    return None
                psk = kpool.tile([128, EMB], f32, tag="psk")
                psq = qpool.tile([128, EMB], f32, tag="psq")
                for ps, xn, wn in ((psk, "k", "wk"), (psq, "q", "wq")):
                    for kc in range(KC):
                        nc.tensor.matmul(
                            ps[:, :L], x_ap(xs, xn, kc, j), w_ap(wn, kc, c),
                            start=(kc == 0), stop=(kc == KC - 1),
                        )
                # DVE may read at most one PSUM operand: stage K through SBUF
                k_sb = wpool.tile([128, EMB], f32, tag="k_sb")
                nc.scalar.copy(k_sb[:, :L], psk[:, :L])
                a = wpool.tile([128, EMB], f32, tag="a")
                nc.vector.tensor_mul(a[:, :L], psq[:, :L], k_sb[:, :L])
                mneg = spool.tile([128, H], f32, tag="mneg")
                av = a[:, :L].rearrange("p (g j) -> p g j", j=c)
                nc.vector.tensor_reduce(mneg[:], av, axis=AX.X, op=OP.max,
                                        negate=True)
                t_m = wpool.tile([128, EMB], f32, tag="t_m")
                mneg_b = (mneg[:].rearrange("p (g o) -> p g o", o=1)
                          .broadcast_to((128, H, c)))
                nc.gpsimd.tensor_add(
                    t_m[:, :L].rearrange("p (g j) -> p g j", j=c), av, mneg_b)
                e = lpool.tile([128, EMB], bf16, tag="e")
                nc.scalar.activation(e[:, :L], t_m[:, :L], ACTF.Exp)
                v_sb = lpool.tile([128, EMB], f16, tag="v_sb")
                nc.scalar.activation(
                    v_sb[:, :L], psv[:, :L], ACTF.Copy,
                    scale=qm_sb[:, j:j + 1],
                )
                return e, v_sb

            def back(j, e, v_sb):
                s = slots[j]
                c, L = s["c"], s["L"]
                ev = e[:, :L].rearrange("p (g j) -> p g j", j=c)
                ssum = spool.tile([128, H], f32, tag="ssum")
                nc.vector.tensor_reduce(ssum[:], ev, axis=AX.X, op=OP.add)
                r = spool.tile([128, H], bf16, tag="r")
                with nc.allow_low_precision(reason="1/S at bf16: ~0.4% on softmax weights, well under the 2e-2 gate"):
                    nc.vector.reciprocal(r[:], ssum[:])
                p = wpool.tile([128, EMB], bf16, tag="p")
                r_b = (r[:].rearrange("p (g o) -> p g o", o=1)
                       .broadcast_to((128, H, c)))
                nc.gpsimd.tensor_mul(
                    p[:, :L].rearrange("p (g j) -> p g j", j=c), ev, r_b)
                nc.vector.tensor_mul(o_slice(j), p[:, :L], v_sb[:, :L])

            done = set()

            def flush(j):
                # issue the sup store once every slot of the sup is issued
                done.add(j)
                sp = j // SUP
                if sp in otiles and all(k in done
                                        for k in range(sup_lo[sp], sup_hi[sp])):
                    store_sup(sp)
                    del otiles[sp]

            xs_cur = load_sup(0, split=2)
            pending = None
            for j in range(nslot + 1):
                if j < nslot:
                    sup, local = divmod(j, SUP)
                    if local == 0 and sup > 0:
                        xs_cur = load_sup(sup)
                    res = front(j, xs_cur)
                else:
                    res = None
                if pending is not None:
                    back(pending[0], *pending[1])
                    flush(pending[0])
                    pending = None
                if j < nslot:
                    if res is None:
                        flush(j)  # vonly output written in front
                    else:
                        pending = (j, res)

    nc.finalize()
    return nc


def _prep_inputs(Q_seq, K_seq, V_seq, Q_len, V_len, WQ, WK, WV):
    slots, assign, total_L = _plan(Q_len, V_len)
    f16 = np.float16
    bf = ml_dtypes.bfloat16
    nslot = len(slots)

    wq_h = np.ascontiguousarray((WQ * 0.125).astype(f16))
    wk_h = np.ascontiguousarray(WK.astype(f16))
    wv_h = np.ascontiguousarray(WV.astype(f16))

    # per-batch transposed fp16 inputs, shared across cores
    qT = {}
    kT = {}
    vT = {}
    for s in slots:
        b = s["b"]
        if b not in vT:
            vT[b] = np.ascontiguousarray(V_seq[b].T.astype(f16))
            if s["kind"] == "reg" or any(
                    t["b"] == b and t["kind"] == "reg" for t in slots):
                qT[b] = np.ascontiguousarray(Q_seq[b].T.astype(f16))
                kT[b] = np.ascontiguousarray(K_seq[b].T.astype(f16))

    in_maps = []
    for i in range(NCORES):
        qg = np.zeros((EMB, nslot * 128), f16)
        kg = np.zeros((EMB, nslot * 128), f16)
        vg = np.zeros((EMB, nslot * 128), f16)
        qmv = np.zeros((128, nslot), np.float32)
        for j, s in enumerate(slots):
            tok0 = assign[i][j]
            if tok0 is None:
                continue
            b = s["b"]
            cs = slice(j * 128, (j + 1) * 128)
            ts = slice(tok0, tok0 + 128)
            vg[:, cs] = vT[b][:, ts]
            ql = int(Q_len[b, 0])
            live = np.clip(ql - tok0, 0, 128)
            scale = (1.0 / 64) if s["kind"] == "vonly" else 1.0
            qmv[:live, j] = scale
            if s["kind"] == "reg":
                qg[:, cs] = qT[b][:, ts]
                kg[:, cs] = kT[b][:, ts]
        in_maps.append({
            "qg": qg, "kg": kg, "vg": vg,
            "wq": wq_h, "wk": wk_h, "wv": wv_h,
            "qm": np.ascontiguousarray(qmv),
        })
    return in_maps, slots, assign, total_L


def _run(inputs, trace=False, mm_dtype_name="", tmpdir=None):
    from concourse.bass_utils import run_bass_kernel_spmd

    Q_len = np.asarray(inputs["Q_len"])
    V_len = np.asarray(inputs["V_len"])
    in_maps, slots, assign, total_L = _prep_inputs(
        np.asarray(inputs["Q_seq"]), np.asarray(inputs["K_seq"]),
        np.asarray(inputs["V_seq"]), Q_len, V_len,
        np.asarray(inputs["WQ"]), np.asarray(inputs["WK"]),
        np.asarray(inputs["WV"]))

    key = tuple((s["kind"], s["L"]) for s in slots)
    if key not in _CACHE:
        _CACHE[key] = _build(slots, total_L)
    nc = _CACHE[key]

    res = run_bass_kernel_spmd(nc, in_maps, core_ids=list(range(NCORES)),
                               trace=trace, tmpdir=tmpdir)

    out = np.zeros((B, S, H * D), np.float32)
    for i in range(NCORES):
        po = res.results[i]["outp"].astype(np.float32)
        for j, s in enumerate(slots):
            tok0 = assign[i][j]
            if tok0 is None:
                continue
            b, c, L, off = s["b"], s["c"], s["L"], s["off"]
            block = po[:, off:off + L].reshape(128, H, c)
            out[b, tok0:tok0 + 128].reshape(128, H, D)[:, :, :c] = block
    return out, res


def kernel(Q_seq, K_seq, V_seq, Q_len, V_len, WQ, WK, WV):
    out, _ = _run(dict(Q_seq=Q_seq, K_seq=K_seq, V_seq=V_seq,
                       Q_len=Q_len, V_len=V_len, WQ=WQ, WK=WK, WV=WV))
    return out


# revision 14
# speedup vs baseline: 1.9279x; 1.1046x over previous
"""Trainium2 Bass kernel for nn_Attention_558345749040.

Reference (per batch b, H=8 heads of d=64, S=4096, E=512):
    Q = Q_seq @ WQ ; K = K_seq @ WK ; V = V_seq @ WV
    A = (Q * K) / 8                      (elementwise)
    softmax over each head's 64-wide feature group, positions j >= V_len[b]
    masked out (V_len == 0 degenerates to a uniform 1/64 softmax)
    O = softmax * V, rows s >= Q_len[b] zeroed

Structure exploited (all derived from the runtime Q_len / V_len values, so
the compiled schedule is input-shape-specialized but value-generic):
  * Rows s >= Q_len[b] are zero: only ceil(Q_len/128) 128-token chunks per
    batch carry live data. Live chunks are repartitioned evenly across the
    8 cores (token-balanced data parallel), removing the Q_len imbalance.
  * Only head positions j < V_len[b] matter: the Q/K/V matmuls select the
    8*V_len live weight columns through a strided moving AP over the shared
    full weight tiles (PE matmul cost scales with output free size), the
    softmax runs on vl-wide groups, and only packed columns are stored; the
    host scatters them back into a zero canvas. Full-quota slots need no
    masking at all; remainder chunks share mixed-width slots and get an
    additive -1e4 pre-softmax mask (fused multiply-add, one DVE op).
  * V_len == 0 batches reduce to O = V/64: V-matmul-only slots.
  * fp16 transport + fp16 matmuls throughout (measured rel err 3.6e-3 vs
    the 2e-2 gate; bf16 Q/K fails at 2.5e-2, fp8 V fails at 3.7e-2).
  * HWDGE DMA triggers cost a flat ~625ns on one shared generator: loads
    are batched 4-contraction-chunks-per-trigger, stores one per superslot
    (issued from the ACT queue so input loads never queue behind them).
  * The Q_len row mask rides the softmax-weight multiply as a per-partition
    scalar (fused (e*qm)*r), so V flows from PSUM straight into the final
    elementwise multiply with no staging copy.

Every core runs the same instruction stream (SPMD single-NEFF constraint):
the slot schedule (widths/kinds) is identical across cores; which batch
chunk a slot processes is pure data (gathered inputs + per-slot masks).
"""

import numpy as np
import ml_dtypes

B, S, EMB = 8, 4096, 512
H, D = 8, 64
NCORES = 8
KC = EMB // 128          # 4 contraction chunks
SUP = 8                  # slots per input-DMA superslot

_CACHE = {}


def _plan(Q_len, V_len):
    """Slot schedule shared by all cores + per-core chunk assignment.

    Returns (slots, assign, total_L, mix_L) where slots[j] holds
    {kind: 'reg'|'mix'|'vonly', c, L, off, moff} and assign[i][j] is
    (batch, tok0) for the chunk core i processes in slot j (None = dummy).
    """
    entries = []  # (slotdict, percore list)

    def chunks_of(b):
        ql = int(Q_len[b, 0])
        return -(-ql // 128) if ql > 0 else 0

    rem = []
    for b in range(B):
        nch = chunks_of(b)
        if nch == 0:
            continue
        vl = int(V_len[b, 0])
        if vl == 0:
            quota = -(-nch // NCORES)
            for t in range(quota):
                per = [(b, (t * NCORES + i) * 128)
                       if t * NCORES + i < nch else None
                       for i in range(NCORES)]
                entries.append(({"kind": "vonly", "c": D, "L": 8 * D}, per))
        else:
            fq = nch // NCORES
            for t in range(fq):
                per = [(b, (t * NCORES + i) * 128) for i in range(NCORES)]
                entries.append(({"kind": "reg", "c": vl, "L": 8 * vl}, per))
            for ch in range(NCORES * fq, nch):
                rem.append((vl, b, ch))

    # Remainder chunks: sort by width desc, deal round-robin into mixed
    # slots whose width is the max of their 8 chunks (others get a -1e4
    # additive mask on the padding columns).
    rem.sort(key=lambda x: -x[0])
    for m in range(0, len(rem), NCORES):
        grp = rem[m:m + NCORES]
        c = grp[0][0]
        per = [(g[1], g[2] * 128) for g in grp]
        per += [None] * (NCORES - len(per))
        entries.append(({"kind": "mix", "c": c, "L": 8 * c}, per))

    # vonly first (need only WV + V data: instant start), then wide->narrow
    # so the pipeline tail drains on cheap slots.
    entries.sort(key=lambda e: (0 if e[0]["kind"] == "vonly" else 1,
                                -e[0]["L"]))

    slots = [e[0] for e in entries]
    assign = [[e[1][i] for e in entries] for i in range(NCORES)]
    off = 0
    moff = 0
    for s in slots:
        s["off"] = off
        off += s["L"]
        if s["kind"] == "mix":
            s["moff"] = moff
            moff += s["L"]
    return slots, assign, off, moff


def _build(slots, total_L, mix_L):
    import concourse.bacc as bacc
    import concourse.mybir as mybir
    from concourse.tile import TileContext

    f32 = mybir.dt.float32
    f16 = mybir.dt.float16
    bf16 = mybir.dt.bfloat16
    AX = mybir.AxisListType
    OP = mybir.AluOpType
    ACTF = mybir.ActivationFunctionType

    nslot = len(slots)
    nvon = sum(1 for s in slots if s["kind"] == "vonly")

    nc = bacc.Bacc()

    qg = nc.declare_dram_parameter("qg", [EMB, nslot * 128], f16, isOutput=False)
    kg = nc.declare_dram_parameter("kg", [EMB, nslot * 128], f16, isOutput=False)
    vg = nc.declare_dram_parameter("vg", [EMB, nslot * 128], f16, isOutput=False)
    wq = nc.declare_dram_parameter("wq", [EMB, EMB], f16, isOutput=False)
    wk = nc.declare_dram_parameter("wk", [EMB, EMB], f16, isOutput=False)
    wv = nc.declare_dram_parameter("wv", [EMB, EMB], f16, isOutput=False)
    qm = nc.declare_dram_parameter("qm", [128, nslot], f32, isOutput=False)
    pm = (nc.declare_dram_parameter("pm", [128, mix_L], bf16, isOutput=False)
          if mix_L else None)
    outp = nc.declare_dram_parameter("outp", [128, total_L], bf16, isOutput=True)

    nsup = -(-nslot // SUP)
    SUPW = SUP * 128

    with TileContext(nc) as tc:
        with (
            tc.tile_pool(name="consts", bufs=1) as cpool,
            tc.tile_pool(name="xin", bufs=2) as xpool,
            tc.tile_pool(name="psq2", bufs=2, space="PSUM") as qpool,
            tc.tile_pool(name="psk2", bufs=2, space="PSUM") as kpool,
            tc.tile_pool(name="psv4", bufs=4, space="PSUM") as vpool,
            tc.tile_pool(name="work", bufs=3) as wpool,
            tc.tile_pool(name="live", bufs=4) as lpool,
            tc.tile_pool(name="outs", bufs=2) as opool,
            tc.tile_pool(name="stats", bufs=4) as spool,
        ):
            qm_sb = cpool.tile([128, nslot], f32, tag="qm")
            nc.sync.dma_start(out=qm_sb[:], in_=qm[:, :])

            w_sb = {}

            def load_w(name, src):
                t = cpool.tile([128, KC * EMB], f16, tag=name, name=name)
                nc.sync.dma_start(
                    out=t[:].rearrange("p (k c) -> p k c", k=KC),
                    in_=src[:, :].rearrange("(k p) c -> p k c", p=128),
                )
                w_sb[name] = t

            def w_ap(name, kc, c):
                blk = w_sb[name][:, kc * EMB:(kc + 1) * EMB]
                if c == 64:
                    return blk
                return blk.rearrange("p (h j) -> p h j", j=D)[:, :, :c]

            def load_sup_piece(tiles, sup, name, src, s0, s1):
                # load slots [s0, s1) of this sup for one tensor: 1 trigger
                lo = sup * SUP
                cols = (s1 - s0) * 128
                tcol = (s0 - lo) * 128
                dst = tiles[name][:].rearrange(
                    "p (k c) -> p k c", k=KC)[:, :, tcol:tcol + cols]
                nc.sync.dma_start(
                    out=dst,
                    in_=src[:, s0 * 128:s1 * 128]
                    .rearrange("(k p) c -> p k c", p=128),
                )

            def sup_tiles():
                return {name: xpool.tile([128, KC * SUPW], f16,
                                         tag=f"x{name}", name=f"x{name}")
                        for name in ("v", "k", "q")}

            def load_sup(sup):
                lo, hi = sup * SUP, min((sup + 1) * SUP, nslot)
                tiles = sup_tiles()
                load_sup_piece(tiles, sup, "v", vg, lo, hi)
                if hi > nvon:
                    load_sup_piece(tiles, sup, "k", kg, max(lo, nvon), hi)
                    load_sup_piece(tiles, sup, "q", qg, max(lo, nvon), hi)
                return tiles

            def x_ap(tiles, name, kc, j):
                col = kc * SUPW + (j % SUP) * 128
                return tiles[name][:, col:col + 128]

            # Startup-critical order: WV + vonly V data first (vonly slots
            # start computing ~3us in), then Q/K weights + the rest.
            load_w("wv", wv)
            xs0 = sup_tiles()
            hi0 = min(SUP, nslot)
            if nvon:
                load_sup_piece(xs0, 0, "v", vg, 0, nvon)
            load_w("wk", wk)
            load_w("wq", wq)
            if hi0 > nvon:
                load_sup_piece(xs0, 0, "v", vg, nvon, hi0)
                load_sup_piece(xs0, 0, "k", kg, nvon, hi0)
                load_sup_piece(xs0, 0, "q", qg, nvon, hi0)
            if pm is not None:
                pm_sb = cpool.tile([128, mix_L], bf16, tag="pm")
                nc.sync.dma_start(out=pm_sb[:], in_=pm[:, :])

            # Output staging: one [128, supL] tile per sup, single store
            # trigger per sup from the ACT queue.
            sup_lo = [sp * SUP for sp in range(nsup)]
            sup_hi = [min((sp + 1) * SUP, nslot) for sp in range(nsup)]
            supL = [sum(slots[j]["L"] for j in range(sup_lo[sp], sup_hi[sp]))
                    for sp in range(nsup)]
            oloc = {}
            for sp in range(nsup):
                col = 0
                for j in range(sup_lo[sp], sup_hi[sp]):
                    oloc[j] = col
                    col += slots[j]["L"]
            max_supL = max(supL)
            otiles = {}

            def o_slice(j):
                sp = j // SUP
                if sp not in otiles:
                    otiles[sp] = opool.tile([128, max_supL], bf16, tag="osup",
                                            name="osup")
                return otiles[sp][:, oloc[j]:oloc[j] + slots[j]["L"]]

            def store_sup(sp):
                g0 = slots[sup_lo[sp]]["off"]
                nc.scalar.dma_start(out=outp[:, g0:g0 + supL[sp]],
                                    in_=otiles[sp][:, :supL[sp]])

            def front(j, xs):
                s = slots[j]
                c, L, kind = s["c"], s["L"], s["kind"]
                psv = vpool.tile([128, EMB], f32, tag="psv")
                for kc in range(KC):
                    nc.tensor.matmul(
                        psv[:, :L], x_ap(xs, "v", kc, j), w_ap("wv", kc, c),
                        start=(kc == 0), stop=(kc == KC - 1),
                    )
                if kind == "vonly":
                    nc.scalar.activation(
                        o_slice(j), psv[:, :L], ACTF.Copy,
                        scale=qm_sb[:, j:j + 1],
                    )
                    return None
                psk = kpool.tile([128, EMB], f32, tag="psk")
                psq = qpool.tile([128, EMB], f32, tag="psq")
                for ps, xn, wn in ((psk, "k", "wk"), (psq, "q", "wq")):
                    for kc in range(KC):
                        nc.tensor.matmul(
                            ps[:, :L], x_ap(xs, xn, kc, j), w_ap(wn, kc, c),
                            start=(kc == 0), stop=(kc == KC - 1),
                        )
                # DVE may read at most one PSUM operand: stage K via SBUF
                k_sb = wpool.tile([128, EMB], f32, tag="k_sb")
                nc.scalar.copy(k_sb[:, :L], psk[:, :L])
                a = wpool.tile([128, EMB], f32, tag="a")
                nc.vector.tensor_mul(a[:, :L], psq[:, :L], k_sb[:, :L])
                if kind == "mix":
                    moff = s["moff"]
                    am = wpool.tile([128, EMB], f32, tag="am")
                    nc.vector.scalar_tensor_tensor(
                        am[:, :L], pm_sb[:, moff:moff + L], -10000.0,
                        a[:, :L], op0=OP.mult, op1=OP.add,
                    )
                    a = am
                mneg = spool.tile([128, H], f32, tag="mneg")
                av = a[:, :L].rearrange("p (g j) -> p g j", j=c)
                nc.vector.tensor_reduce(mneg[:], av, axis=AX.X, op=OP.max,
                                        negate=True)
                t_m = wpool.tile([128, EMB], f32, tag="t_m")
                mneg_b = (mneg[:].rearrange("p (g o) -> p g o", o=1)
                          .broadcast_to((128, H, c)))
                nc.gpsimd.tensor_add(
                    t_m[:, :L].rearrange("p (g j) -> p g j", j=c), av, mneg_b)
                e = lpool.tile([128, EMB], bf16, tag="e")
                # Q_len row mask rides the exp bias: dead rows get -1e4 so
                # e == 0 there (the resulting 0*inf NaNs in dead rows are
                # zeroed by the host scatter).
                nc.scalar.activation(e[:, :L], t_m[:, :L], ACTF.Exp,
                                     bias=qm_sb[:, j:j + 1])
                return e, psv

            def back(j, e, psv):
                s = slots[j]
                c, L = s["c"], s["L"]
                ev = e[:, :L].rearrange("p (g j) -> p g j", j=c)
                ssum = spool.tile([128, H], f32, tag="ssum")
                nc.vector.tensor_reduce(ssum[:], ev, axis=AX.X, op=OP.add)
                r = spool.tile([128, H], bf16, tag="r")
                with nc.allow_low_precision(reason="1/S at bf16: ~0.4% on softmax weights, well under the 2e-2 gate"):
                    nc.vector.reciprocal(r[:], ssum[:])
                p = wpool.tile([128, EMB], bf16, tag="p")
                r_b = (r[:].rearrange("p (g o) -> p g o", o=1)
                       .broadcast_to((128, H, c)))
                nc.gpsimd.tensor_mul(
                    p[:, :L].rearrange("p (g j) -> p g j", j=c), ev, r_b)
                nc.vector.tensor_mul(o_slice(j), p[:, :L], psv[:, :L])

            done = set()

            def flush(j):
                done.add(j)
                sp = j // SUP
                if sp in otiles and all(k in done
                                        for k in range(sup_lo[sp], sup_hi[sp])):
                    store_sup(sp)
                    del otiles[sp]

            xs_cur = xs0
            pending = None
            for j in range(nslot + 1):
                if j < nslot:
                    sup, local = divmod(j, SUP)
                    if local == 0 and sup > 0:
                        xs_cur = load_sup(sup)
                    res = front(j, xs_cur)
                else:
                    res = None
                if pending is not None:
                    back(pending[0], *pending[1])
                    flush(pending[0])
                    pending = None
                if j < nslot:
                    if res is None:
                        flush(j)
                    else:
                        pending = (j, res)

    nc.finalize()
    return nc


def _prep_inputs(Q_seq, K_seq, V_seq, Q_len, V_len, WQ, WK, WV):
    slots, assign, total_L, mix_L = _plan(Q_len, V_len)
    f16 = np.float16
    bf = ml_dtypes.bfloat16
    nslot = len(slots)

    wq_h = np.ascontiguousarray((WQ * 0.125).astype(f16))
    wk_h = np.ascontiguousarray(WK.astype(f16))
    wv_h = np.ascontiguousarray(WV.astype(f16))

    need_qk = {ba for i in range(NCORES) for j, s in enumerate(slots)
               if s["kind"] != "vonly" and assign[i][j] is not None
               for ba in [assign[i][j][0]]}
    need_v = {ba for i in range(NCORES) for j in range(nslot)
              if assign[i][j] is not None
              for ba in [assign[i][j][0]]}
    qT = {b: np.ascontiguousarray(Q_seq[b].T.astype(f16)) for b in need_qk}
    kT = {b: np.ascontiguousarray(K_seq[b].T.astype(f16)) for b in need_qk}
    vT = {b: np.ascontiguousarray(V_seq[b].T.astype(f16)) for b in need_v}

    in_maps = []
    for i in range(NCORES):
        qg = np.zeros((EMB, nslot * 128), f16)
        kg = np.zeros((EMB, nslot * 128), f16)
        vg = np.zeros((EMB, nslot * 128), f16)
        qmv = np.zeros((128, nslot), np.float32)
        pmv = np.zeros((128, mix_L), bf) if mix_L else None
        for j, s in enumerate(slots):
            ent = assign[i][j]
            if ent is None:
                continue
            b, tok0 = ent
            cs = slice(j * 128, (j + 1) * 128)
            ts = slice(tok0, tok0 + 128)
            vg[:, cs] = vT[b][:, ts]
            ql = int(Q_len[b, 0])
            live = int(np.clip(ql - tok0, 0, 128))
            if s["kind"] == "vonly":
                # multiplicative scale on the V copy (folds the 1/64)
                qmv[:live, j] = 1.0 / 64
            else:
                # additive exp bias: -1e4 on dead rows zeroes e there
                qmv[live:, j] = -1e4
            if s["kind"] != "vonly":
                qg[:, cs] = qT[b][:, ts]
                kg[:, cs] = kT[b][:, ts]
            if s["kind"] == "mix":
                vl = int(V_len[b, 0])
                c = s["c"]
                if vl < c:
                    dead = np.zeros((H, c), np.float32)
                    dead[:, vl:] = 1.0
                    pmv[:, s["moff"]:s["moff"] + s["L"]] = \
                        np.broadcast_to(dead.reshape(-1), (128, s["L"]))
        m = {
            "qg": qg, "kg": kg, "vg": vg,
            "wq": wq_h, "wk": wk_h, "wv": wv_h,
            "qm": np.ascontiguousarray(qmv),
        }
        if mix_L:
            m["pm"] = np.ascontiguousarray(pmv)
        in_maps.append(m)
    return in_maps, slots, assign, total_L


def _run(inputs, trace=False, mm_dtype_name="", tmpdir=None):
    from concourse.bass_utils import run_bass_kernel_spmd

    Q_len = np.asarray(inputs["Q_len"])
    V_len = np.asarray(inputs["V_len"])
    in_maps, slots, assign, total_L = _prep_inputs(
        np.asarray(inputs["Q_seq"]), np.asarray(inputs["K_seq"]),
        np.asarray(inputs["V_seq"]), Q_len, V_len,
        np.asarray(inputs["WQ"]), np.asarray(inputs["WK"]),
        np.asarray(inputs["WV"]))

    key = tuple((s["kind"], s["L"]) for s in slots)
    if key not in _CACHE:
        mix_L = sum(s["L"] for s in slots if s["kind"] == "mix")
        _CACHE[key] = _build(slots, total_L, mix_L)
    nc = _CACHE[key]

    res = run_bass_kernel_spmd(nc, in_maps, core_ids=list(range(NCORES)),
                               trace=trace, tmpdir=tmpdir)

    out = np.zeros((B, S, H * D), np.float32)
    for i in range(NCORES):
        po = res.results[i]["outp"].astype(np.float32)
        for j, s in enumerate(slots):
            ent = assign[i][j]
            if ent is None:
                continue
            b, tok0 = ent
            c, L, off = s["c"], s["L"], s["off"]
            live = int(np.clip(int(Q_len[b, 0]) - tok0, 0, 128))
            block = po[:live, off:off + L].reshape(live, H, c)
            if s["kind"] == "vonly":
                out[b, tok0:tok0 + live] = block.reshape(live, H * D)
            else:
                vl = int(V_len[b, 0])
                out[b, tok0:tok0 + live].reshape(live, H, D)[:, :, :vl] = \
                    block[:, :, :vl]
    return out, res


def kernel(Q_seq, K_seq, V_seq, Q_len, V_len, WQ, WK, WV):
    out, _ = _run(dict(Q_seq=Q_seq, K_seq=K_seq, V_seq=V_seq,
                       Q_len=Q_len, V_len=V_len, WQ=WQ, WK=WK, WV=WV))
    return out
